# revision 1
# baseline (speedup 1.0000x reference)
"""LoRA layer kernel for Trainium2, 8-core data-parallel.

out = x @ W.T + 2.0 * ((x @ B) @ A)
  x: (4, 4096, 4096) f32, W: (4096, 4096), A: (16, 4096), B: (4096, 16)

Strategy: flatten x to (16384, 4096) rows, shard rows across 8 cores
(2048 rows each), replicate W/A/B. Per core a single fused GEMM:
  - x-block stationary (fp32r), W.T streamed as moving operand
  - LoRA: tT = (x @ B).T computed per block (contraction over full K),
    then one extra K=16 matmul per (m_tile, o_chunk) accumulates
    2*(x@B)@A into the same PSUM bank (A pre-scaled by 2 on host).
All device matmuls use float32r: 1 cycle/row at N=512 (same rate as
bf16, ~TF32 precision).
"""

import sys

if "/opt/trn_rl_repo" not in sys.path:
    sys.path.insert(0, "/opt/trn_rl_repo")

import os

import numpy as np

import concourse.bass as bass
import concourse.mybir as mybir
import concourse.tile as tile

N_CORES = 8
D = 4096
RANK = 16
ROWS_TOTAL = 4 * 4096          # 16384
ROWS_PER_CORE = ROWS_TOTAL // N_CORES  # 2048
P = 128
KT = D // P                    # 32 k-tiles
M_BLOCK = 1024                 # rows per x-resident block
N_BLOCKS = ROWS_PER_CORE // M_BLOCK    # 2
MT_PER_BLOCK = M_BLOCK // P    # 8 m-tiles (PSUM banks)
OC = 512                       # o-chunk width (one PSUM bank)
N_OC = D // OC                 # 8
KH = KT // 2                   # k-tiles per x half-tile

F32 = mybir.dt.float32
F32R = mybir.dt.float32r

W_PAIR = os.environ.get("K_WPAIR", "1") == "1"
GP_DMA = os.environ.get("K_GPDMA", "1") == "1"
WARMUP = os.environ.get("K_WARMUP", "1") == "1"


def _dma_gp(nc):
    return nc.gpsimd if GP_DMA else nc.sync


def split_wide_waits(nc, max_waits=1):
    """walrus in this container rejects >1 sync wait per instruction;
    move excess waits onto preceding same-engine NoOps."""
    n_split = 0
    for f in nc.m.functions:
        for bb in f.blocks:
            new_insts = []
            for inst in bb.instructions:
                si = getattr(inst, "sync_info", None)
                if si is not None and si.on_wait and len(si.on_wait) > max_waits:
                    waits = list(si.on_wait)
                    keep = waits[-max_waits:]
                    extra = waits[:-max_waits]
                    for i in range(0, len(extra), max_waits):
                        chunk = extra[i:i + max_waits]
                        nop = mybir.InstNoOp(
                            name=f"{inst.name}_wsplit{i}",
                            sync_info=mybir.SyncInfo(on_wait=chunk, on_update=[]),
                            bass_nofuse=True,
                            engine=inst.engine,
                        )
                        new_insts.append(nop)
                        n_split += 1
                    si.on_wait = keep
                new_insts.append(inst)
            bb.instructions[:] = new_insts
    return n_split


def build_program():
    nc = bass.Bass()
    xt = nc.declare_dram_parameter("xt", [D, ROWS_PER_CORE], F32R, isOutput=False)
    wt = nc.declare_dram_parameter("wt", [D, D], F32R, isOutput=False)
    # bmat pre-arranged on host: [128, KT*RANK], col-block k = rows k*128..+128
    bmat = nc.declare_dram_parameter("bmat", [P, KT * RANK], F32R, isOutput=False)
    a2 = nc.declare_dram_parameter("a2", [RANK, D], F32R, isOutput=False)
    out = nc.declare_dram_parameter("out", [ROWS_PER_CORE, D], F32, isOutput=True)

    with tile.TileContext(nc) as tc:
        with (
            tc.tile_pool(name="xpool_a", bufs=1) as xpool_a,
            tc.tile_pool(name="xpool_b", bufs=1) as xpool_b,
            tc.tile_pool(name="wpool", bufs=6) as wpool,
            tc.tile_pool(name="opool", bufs=4) as opool,
            tc.tile_pool(name="cpool", bufs=1) as cpool,
            tc.tile_pool(name="tpool", bufs=2) as tpool,
            tc.tile_pool(name="ppool", bufs=8, space="PSUM") as ppool,
        ):
            # constants: B (pre-arranged) and A2 — single DMAs on gpsimd queue
            btile = cpool.tile([P, KT * RANK], F32R, tag="bt")
            _dma_gp(nc).dma_start(btile[:], bmat[:])
            atile = cpool.tile([RANK, D], F32R, tag="at")
            _dma_gp(nc).dma_start(atile[:], a2[:])

            # HAM warmup: ~5us of dummy matmuls so the PE clock is at 8/8
            # before real work lands (3.4us busy window un-throttles).
            if WARMUP:
                junk = ppool.tile([RANK, OC], F32, tag="acc", name="junk")
                for i in range(25):
                    nc.tensor.matmul(
                        junk[:],
                        btile[:, :RANK],
                        btile[:, :OC],
                        start=(i == 0),
                        stop=(i == 24),
                    )

            for blk in range(N_BLOCKS):
                r0 = blk * M_BLOCK
                # x block resident: two half tiles (k 0-15, k 16-31)
                xa = xpool_a.tile([P, KH * M_BLOCK], F32R, tag="xa")
                xb = xpool_b.tile([P, KH * M_BLOCK], F32R, tag="xb")

                def xsl(k, c0, cw):
                    t = xa if k < KH else xb
                    kk = k % KH
                    return t[:, kk * M_BLOCK + c0: kk * M_BLOCK + c0 + cw]

                for k in range(KT):
                    eng = (nc.gpsimd if k % 2 == 0 else nc.scalar) if GP_DMA else nc.sync
                    eng.dma_start(
                        xsl(k, 0, M_BLOCK),
                        xt[k * P:(k + 1) * P, r0:r0 + M_BLOCK],
                    )

                # stage A: tT[r, m] = sum_i B[i,r] * x[m,i]  (per block)
                tT = tpool.tile([RANK, M_BLOCK], F32R, tag="tT")
                for h in range(M_BLOCK // OC):
                    pt = ppool.tile([RANK, OC], F32, tag="acc")
                    for k in range(KT):
                        nc.tensor.matmul(
                            pt[:],
                            btile[:, k * RANK:(k + 1) * RANK],
                            xsl(k, h * OC, OC),
                            start=(k == 0),
                            stop=(k == KT - 1),
                        )
                    nc.vector.tensor_copy(tT[:, h * OC:(h + 1) * OC], pt[:])

                # main GEMM + fused LoRA accumulation.
                # W fetched as adjacent k-tile pairs [128, 2*OC] (halves the
                # ~0.6us/DMA issue count on the sync queue).
                for oc in range(N_OC):
                    psums = []
                    for mt in range(MT_PER_BLOCK):
                        psums.append(ppool.tile([P, OC], F32, tag="acc", name=f"ps_{blk}_{oc}_{mt}"))
                    for k2 in range(KT // 2):
                        wtile = wpool.tile([P, 2 * OC], F32R, tag="wt")
                        src = wt[k2 * 2 * P:(k2 + 1) * 2 * P,
                                 oc * OC:(oc + 1) * OC]
                        if W_PAIR:
                            nc.sync.dma_start(
                                wtile.rearrange("p (b c) -> p b c", b=2),
                                src.rearrange("(b p) c -> p b c", p=P),
                            )
                        else:
                            for half in range(2):
                                nc.sync.dma_start(
                                    wtile[:, half * OC:(half + 1) * OC],
                                    wt[(2 * k2 + half) * P:(2 * k2 + half + 1) * P,
                                       oc * OC:(oc + 1) * OC],
                                )
                        for half in range(2):
                            k = 2 * k2 + half
                            for mt in range(MT_PER_BLOCK):
                                nc.tensor.matmul(
                                    psums[mt][:],
                                    xsl(k, mt * P, P),
                                    wtile[:, half * OC:(half + 1) * OC],
                                    start=(k == 0),
                                    stop=False,
                                )
                    for mt in range(MT_PER_BLOCK):
                        # LoRA: += tT[:, mt].T @ (2A[:, oc])
                        nc.tensor.matmul(
                            psums[mt][:],
                            tT[:, mt * P:(mt + 1) * P],
                            atile[:, oc * OC:(oc + 1) * OC],
                            start=False,
                            stop=True,
                        )
                        ot = opool.tile([P, OC], F32, tag="ot")
                        nc.vector.tensor_copy(ot[:], psums[mt][:])
                        nc.sync.dma_start(
                            out[r0 + mt * P:r0 + (mt + 1) * P,
                                oc * OC:(oc + 1) * OC],
                            ot[:],
                        )

    split_wide_waits(nc)
    return nc


_NC_CACHE = [None]


def kernel(x, weight, lora_A, lora_B):
    from concourse.bass_utils import run_bass_kernel_spmd

    x = np.asarray(x, dtype=np.float32)
    weight = np.asarray(weight, dtype=np.float32)
    lora_A = np.asarray(lora_A, dtype=np.float32)
    lora_B = np.asarray(lora_B, dtype=np.float32)

    x2 = x.reshape(ROWS_TOTAL, D)
    wt = np.ascontiguousarray(weight.T)
    a2 = np.ascontiguousarray(2.0 * lora_A)
    # pre-arrange B: [128, KT*RANK], col-block k holds rows k*128..(k+1)*128
    bmat = np.ascontiguousarray(
        lora_B.reshape(KT, P, RANK).transpose(1, 0, 2).reshape(P, KT * RANK)
    )

    in_maps = []
    for c in range(N_CORES):
        xt_c = np.ascontiguousarray(
            x2[c * ROWS_PER_CORE:(c + 1) * ROWS_PER_CORE].T
        )
        in_maps.append({"xt": xt_c, "wt": wt, "bmat": bmat, "a2": a2})

    if _NC_CACHE[0] is None:
        _NC_CACHE[0] = build_program()
    nc = _NC_CACHE[0]

    res = run_bass_kernel_spmd(nc, in_maps, list(range(N_CORES)))
    out = np.concatenate(
        [res.results[c]["out"] for c in range(N_CORES)], axis=0
    )
    return out.reshape(x.shape)



# revision 2
# speedup vs baseline: 1.1759x; 1.1759x over previous
"""LoRA layer kernel for Trainium2, 8-core data-parallel.

out = x @ W.T + 2.0 * ((x @ B) @ A)
  x: (4, 4096, 4096) f32, W: (4096, 4096), A: (16, 4096), B: (4096, 16)

Strategy: fold the LoRA path into the weight on the host
(W'' = W.T + 2*B@A, a 0.5-GFLOP rank-16 update), so the device runs a
single pure GEMM per core:  out_c[2048, 4096] = x_c[2048, 4096] @ W''.
Rows are sharded across the 8 cores (2048 each), W'' replicated.

Per-core kernel (bf16 inputs, fp32 PSUM accumulate):
  - x shard fully SBUF-resident as 64 tiles [128, 1024] bf16 (128 KiB/par)
  - loop oc(8 chunks of 512) x mg(2 row groups of 1024) x k(32):
    8 matmuls [128k,128m]x[128k,512n] accumulating into 8 PSUM banks
  - W'' oc-slice (32 tiles [128,512] bf16) loaded once per oc, reused by
    both row groups; wpool bufs=40 gives ~8 tiles of cross-oc prefetch
  - k-ascending first pass consumes x tiles as their DMAs land, so the
    startup stall is one 256 KiB tile, not the full 16 MiB shard
  - short junk-matmul warmup trips the HAM clock gate (1.2->2.4 GHz)
    while the first x tiles are still in flight
"""

import sys

if "/opt/trn_rl_repo" not in sys.path:
    sys.path.insert(0, "/opt/trn_rl_repo")

import numpy as np

import concourse.bass as bass
import concourse.mybir as mybir
import concourse.tile as tile

N_CORES = 8
D = 4096
ROWS_TOTAL = 4 * 4096              # 16384
ROWS_PER_CORE = ROWS_TOTAL // N_CORES  # 2048
P = 128
KT = D // P                        # 32 k-tiles
MG = 2                             # row groups per core
MG_ROWS = ROWS_PER_CORE // MG      # 1024
MT = MG_ROWS // P                  # 8 m-tiles (PSUM banks) per group
OC = 512                           # o-chunk width (one PSUM bank)
N_OC = D // OC                     # 8

F32 = mybir.dt.float32
BF16 = mybir.dt.bfloat16

N_WARMUP = 10


def split_wide_waits(nc, max_waits=1):
    """walrus in this container rejects >1 sync wait per instruction;
    move excess waits onto preceding same-engine NoOps."""
    n_split = 0
    for f in nc.m.functions:
        for bb in f.blocks:
            new_insts = []
            for inst in bb.instructions:
                si = getattr(inst, "sync_info", None)
                if si is not None and si.on_wait and len(si.on_wait) > max_waits:
                    waits = list(si.on_wait)
                    keep = waits[-max_waits:]
                    extra = waits[:-max_waits]
                    for i in range(0, len(extra), max_waits):
                        chunk = extra[i:i + max_waits]
                        nop = mybir.InstNoOp(
                            name=f"{inst.name}_wsplit{i}",
                            sync_info=mybir.SyncInfo(on_wait=chunk, on_update=[]),
                            bass_nofuse=True,
                            engine=inst.engine,
                        )
                        new_insts.append(nop)
                        n_split += 1
                    si.on_wait = keep
                new_insts.append(inst)
            bb.instructions[:] = new_insts
    return n_split


def build_program():
    nc = bass.Bass()
    xt = nc.declare_dram_parameter("xt", [D, ROWS_PER_CORE], BF16, isOutput=False)
    wt = nc.declare_dram_parameter("wt", [D, D], BF16, isOutput=False)
    out = nc.declare_dram_parameter("out", [ROWS_PER_CORE, D], F32, isOutput=True)

    with tile.TileContext(nc) as tc:
        with (
            tc.tile_pool(name="xpool", bufs=KT * MG) as xpool,
            tc.tile_pool(name="wpool", bufs=40) as wpool,
            tc.tile_pool(name="opool", bufs=6) as opool,
            tc.tile_pool(name="wupool", bufs=1) as wupool,
            tc.tile_pool(name="ppool", bufs=8, space="PSUM") as ppool,
        ):
            # HAM warmup: junk matmuls trip the PE clock gate to 8/8
            # while the first x tiles stream in.
            wu = wupool.tile([P, OC], BF16, tag="wu")
            nc.vector.memset(wu[:], 0.0)
            junk = ppool.tile([P, OC], F32, tag="acc", name="junk")
            for i in range(N_WARMUP):
                nc.tensor.matmul(
                    junk[:],
                    wu[:, :P],
                    wu[:],
                    start=(i == 0),
                    stop=(i == N_WARMUP - 1),
                )

            # x shard resident: 64 tiles [128, 1024], issued in the order
            # the first pass consumes them (mg-major, k-ascending).
            xtiles = [[None] * KT for _ in range(MG)]
            for mg in range(MG):
                for k in range(KT):
                    t = xpool.tile([P, MG_ROWS], BF16, tag="x")
                    eng = nc.gpsimd if k % 2 == 0 else nc.scalar
                    eng.dma_start(
                        t[:],
                        xt[k * P:(k + 1) * P,
                           mg * MG_ROWS:(mg + 1) * MG_ROWS],
                    )
                    xtiles[mg][k] = t

            for oc in range(N_OC):
                wtiles = [None] * KT
                for mg in range(MG):
                    psums = [
                        ppool.tile([P, OC], F32, tag="acc",
                                   name=f"ps_{oc}_{mg}_{mt}")
                        for mt in range(MT)
                    ]
                    for k in range(KT):
                        if mg == 0:
                            w = wpool.tile([P, OC], BF16, tag="wt")
                            nc.sync.dma_start(
                                w[:],
                                wt[k * P:(k + 1) * P, oc * OC:(oc + 1) * OC],
                            )
                            wtiles[k] = w
                        xk = xtiles[mg][k]
                        for mt in range(MT):
                            nc.tensor.matmul(
                                psums[mt][:],
                                xk[:, mt * P:(mt + 1) * P],
                                wtiles[k][:],
                                start=(k == 0),
                                stop=(k == KT - 1),
                            )
                    for mt in range(MT):
                        ot = opool.tile([P, OC], F32, tag="ot")
                        nc.vector.tensor_copy(ot[:], psums[mt][:])
                        nc.scalar.dma_start(
                            out[mg * MG_ROWS + mt * P:
                                mg * MG_ROWS + (mt + 1) * P,
                                oc * OC:(oc + 1) * OC],
                            ot[:],
                        )

    split_wide_waits(nc)
    return nc


_NC_CACHE = [None]


def kernel(x, weight, lora_A, lora_B):
    import ml_dtypes
    from concourse.bass_utils import run_bass_kernel_spmd

    bf16 = ml_dtypes.bfloat16

    x = np.asarray(x, dtype=np.float32)
    weight = np.asarray(weight, dtype=np.float32)
    lora_A = np.asarray(lora_A, dtype=np.float32)
    lora_B = np.asarray(lora_B, dtype=np.float32)

    # fold LoRA: out = x @ (W.T + 2*B@A)
    wfold = weight.T + 2.0 * (lora_B @ lora_A)
    wt = np.ascontiguousarray(wfold.astype(bf16))

    x2 = x.reshape(ROWS_TOTAL, D)
    xt_all = x2.T.astype(bf16)     # [D, ROWS_TOTAL] C-contiguous

    in_maps = []
    for c in range(N_CORES):
        xt_c = np.ascontiguousarray(
            xt_all[:, c * ROWS_PER_CORE:(c + 1) * ROWS_PER_CORE]
        )
        in_maps.append({"xt": xt_c, "wt": wt})

    if _NC_CACHE[0] is None:
        _NC_CACHE[0] = build_program()
    nc = _NC_CACHE[0]

    res = run_bass_kernel_spmd(nc, in_maps, list(range(N_CORES)))
    out = np.concatenate(
        [res.results[c]["out"] for c in range(N_CORES)], axis=0
    )
    return out.reshape(x.shape)


# revision 6
# speedup vs baseline: 1.1983x; 1.0190x over previous
"""LoRA layer kernel for Trainium2, 8-core data-parallel.

out = x @ W.T + 2.0 * ((x @ B) @ A)
  x: (4, 4096, 4096) f32, W: (4096, 4096), A: (16, 4096), B: (4096, 16)

Strategy: fold the LoRA path into the weight on the host
(W'' = W.T + 2*B@A, a 0.5-GFLOP rank-16 update), so the device runs a
single pure GEMM per core:  out_c[2048, 4096] = x_c[2048, 4096] @ W''.
Rows are sharded across the 8 cores (2048 each), W'' replicated.

Per-core kernel (bf16 inputs, fp32 PSUM accumulate):
  - x shard fully SBUF-resident as 64 tiles [128, 1024] bf16 (128 KiB/par)
  - loop oc(8 chunks of 512) x mg(2 row groups of 1024) x k(32):
    8 matmuls [128k,128m]x[128k,512n] accumulating into 8 PSUM banks
  - W'' oc-slice (32 tiles [128,512] bf16) loaded once per oc, reused by
    both row groups; wpool bufs=40 gives ~8 tiles of cross-oc prefetch
  - k-ascending first pass consumes x tiles as their DMAs land, so the
    startup stall is one 256 KiB tile, not the full 16 MiB shard
  - short junk-matmul warmup trips the HAM clock gate (1.2->2.4 GHz)
    while the first x tiles are still in flight
"""

import sys

if "/opt/trn_rl_repo" not in sys.path:
    sys.path.insert(0, "/opt/trn_rl_repo")

import numpy as np

import concourse.bass as bass
import concourse.mybir as mybir
import concourse.tile as tile

N_CORES = 8
D = 4096
ROWS_TOTAL = 4 * 4096              # 16384
ROWS_PER_CORE = ROWS_TOTAL // N_CORES  # 2048
P = 128
KT = D // P                        # 32 k-tiles
MG = 2                             # row groups per core
MG_ROWS = ROWS_PER_CORE // MG      # 1024
MT = MG_ROWS // P                  # 8 m-tiles (PSUM banks) per group
OC = 512                           # o-chunk width (one PSUM bank)
N_OC = D // OC                     # 8

F32 = mybir.dt.float32
BF16 = mybir.dt.bfloat16

N_WARMUP = 10


def split_wide_waits(nc, max_waits=1):
    """walrus in this container rejects >1 sync wait per instruction;
    move excess waits onto preceding same-engine NoOps."""
    n_split = 0
    for f in nc.m.functions:
        for bb in f.blocks:
            new_insts = []
            for inst in bb.instructions:
                si = getattr(inst, "sync_info", None)
                if si is not None and si.on_wait and len(si.on_wait) > max_waits:
                    waits = list(si.on_wait)
                    keep = waits[-max_waits:]
                    extra = waits[:-max_waits]
                    for i in range(0, len(extra), max_waits):
                        chunk = extra[i:i + max_waits]
                        nop = mybir.InstNoOp(
                            name=f"{inst.name}_wsplit{i}",
                            sync_info=mybir.SyncInfo(on_wait=chunk, on_update=[]),
                            bass_nofuse=True,
                            engine=inst.engine,
                        )
                        new_insts.append(nop)
                        n_split += 1
                    si.on_wait = keep
                new_insts.append(inst)
            bb.instructions[:] = new_insts
    return n_split


def build_program():
    nc = bass.Bass()
    xt = nc.declare_dram_parameter("xt", [D, ROWS_PER_CORE], BF16, isOutput=False)
    wt = nc.declare_dram_parameter("wt", [D, D], BF16, isOutput=False)
    out = nc.declare_dram_parameter("out", [ROWS_PER_CORE, D], F32, isOutput=True)

    with tile.TileContext(nc) as tc:
        with (
            tc.tile_pool(name="xpool", bufs=KT * MG) as xpool,
            tc.tile_pool(name="wpool", bufs=36) as wpool,
            tc.tile_pool(name="opool", bufs=12) as opool,
            tc.tile_pool(name="wupool", bufs=1) as wupool,
            tc.tile_pool(name="ppool", bufs=8, space="PSUM") as ppool,
        ):
            # HAM warmup: junk matmuls trip the PE clock gate to 8/8
            # while the first x tiles stream in.
            wu = wupool.tile([P, OC], BF16, tag="wu")
            nc.vector.memset(wu[:], 0.0)
            junk = ppool.tile([P, OC], F32, tag="acc", name="junk")
            for i in range(N_WARMUP):
                nc.tensor.matmul(
                    junk[:],
                    wu[:, :P],
                    wu[:],
                    start=(i == 0),
                    stop=(i == N_WARMUP - 1),
                )

            # x shard resident: 64 tiles [128, 1024], issued in the order
            # the first pass consumes them (mg-major, k-ascending).
            # mg0 goes on the gpsimd/scalar rings; W oc0 is hoisted onto
            # the sync ring first, with mg1's x behind it (per-ring FIFO
            # keeps the mg1 prefetch from stealing HBM bandwidth from
            # pass 0's just-in-time W stream).
            xtiles = [[None] * KT for _ in range(MG)]
            for k in range(KT):
                t = xpool.tile([P, MG_ROWS], BF16, tag="x")
                eng = nc.gpsimd if k % 2 == 0 else nc.scalar
                eng.dma_start(t[:], xt[k * P:(k + 1) * P, 0:MG_ROWS])
                xtiles[0][k] = t

            wtiles0 = [None] * KT
            for k in range(KT):
                w = wpool.tile([P, OC], BF16, tag="wt")
                nc.sync.dma_start(w[:], wt[k * P:(k + 1) * P, 0:OC])
                wtiles0[k] = w

            for k in range(KT):
                t = xpool.tile([P, MG_ROWS], BF16, tag="x")
                nc.sync.dma_start(
                    t[:], xt[k * P:(k + 1) * P, MG_ROWS:2 * MG_ROWS]
                )
                xtiles[1][k] = t

            for oc in range(N_OC):
                wtiles = wtiles0 if oc == 0 else [None] * KT
                for mg in range(MG):
                    psums = [
                        ppool.tile([P, OC], F32, tag="acc",
                                   name=f"ps_{oc}_{mg}_{mt}")
                        for mt in range(MT)
                    ]
                    for k in range(KT):
                        if mg == 0 and oc > 0:
                            w = wpool.tile([P, OC], BF16, tag="wt")
                            nc.sync.dma_start(
                                w[:],
                                wt[k * P:(k + 1) * P, oc * OC:(oc + 1) * OC],
                            )
                            wtiles[k] = w
                        xk = xtiles[mg][k]
                        for mt in range(MT):
                            nc.tensor.matmul(
                                psums[mt][:],
                                xk[:, mt * P:(mt + 1) * P],
                                wtiles[k][:],
                                start=(k == 0),
                                stop=(k == KT - 1),
                            )
                    for mt in range(MT):
                        ot = opool.tile([P, OC], F32, tag="ot")
                        nc.vector.tensor_copy(ot[:], psums[mt][:])
                        # alternate HWDGE rings so the drain chain at
                        # pass boundaries / kernel tail is half as deep
                        eng = nc.scalar if mt % 2 == 0 else nc.sync
                        eng.dma_start(
                            out[mg * MG_ROWS + mt * P:
                                mg * MG_ROWS + (mt + 1) * P,
                                oc * OC:(oc + 1) * OC],
                            ot[:],
                        )

    split_wide_waits(nc)
    return nc


_NC_CACHE = [None]


def kernel(x, weight, lora_A, lora_B):
    import ml_dtypes
    from concourse.bass_utils import run_bass_kernel_spmd

    bf16 = ml_dtypes.bfloat16

    x = np.asarray(x, dtype=np.float32)
    weight = np.asarray(weight, dtype=np.float32)
    lora_A = np.asarray(lora_A, dtype=np.float32)
    lora_B = np.asarray(lora_B, dtype=np.float32)

    # fold LoRA: out = x @ (W.T + 2*B@A)
    wfold = weight.T + 2.0 * (lora_B @ lora_A)
    wt = np.ascontiguousarray(wfold.astype(bf16))

    x2 = x.reshape(ROWS_TOTAL, D)
    xt_all = x2.T.astype(bf16)     # [D, ROWS_TOTAL] C-contiguous

    in_maps = []
    for c in range(N_CORES):
        xt_c = np.ascontiguousarray(
            xt_all[:, c * ROWS_PER_CORE:(c + 1) * ROWS_PER_CORE]
        )
        in_maps.append({"xt": xt_c, "wt": wt})

    if _NC_CACHE[0] is None:
        _NC_CACHE[0] = build_program()
    nc = _NC_CACHE[0]

    res = run_bass_kernel_spmd(nc, in_maps, list(range(N_CORES)))
    out = np.concatenate(
        [res.results[c]["out"] for c in range(N_CORES)], axis=0
    )
    return out.reshape(x.shape)


# revision 9
# speedup vs baseline: 1.2044x; 1.0051x over previous
"""LoRA layer kernel for Trainium2, 8-core data-parallel.

out = x @ W.T + 2.0 * ((x @ B) @ A)
  x: (4, 4096, 4096) f32, W: (4096, 4096), A: (16, 4096), B: (4096, 16)

Strategy: fold the LoRA path into the weight on the host
(W'' = W.T + 2*B@A, a 0.5-GFLOP rank-16 update), so the device runs a
single pure GEMM per core:  out_c[2048, 4096] = x_c[2048, 4096] @ W''.
Rows are sharded across the 8 cores (2048 each), W'' replicated.

Per-core kernel (bf16 inputs, fp32 PSUM accumulate):
  - x shard fully SBUF-resident as 64 tiles [128, 1024] bf16 (128 KiB/par)
  - loop oc(8 chunks of 512) x mg(2 row groups of 1024) x k(32):
    8 matmuls [128k,128m]x[128k,512n] accumulating into 8 PSUM banks
  - W'' oc-slice (32 tiles [128,512] bf16) loaded once per oc, reused by
    both row groups; wpool bufs=40 gives ~8 tiles of cross-oc prefetch
  - k-ascending first pass consumes x tiles as their DMAs land, so the
    startup stall is one 256 KiB tile, not the full 16 MiB shard
  - short junk-matmul warmup trips the HAM clock gate (1.2->2.4 GHz)
    while the first x tiles are still in flight
"""

import sys

if "/opt/trn_rl_repo" not in sys.path:
    sys.path.insert(0, "/opt/trn_rl_repo")

import numpy as np

import concourse.bass as bass
import concourse.mybir as mybir
import concourse.tile as tile

N_CORES = 8
D = 4096
ROWS_TOTAL = 4 * 4096              # 16384
ROWS_PER_CORE = ROWS_TOTAL // N_CORES  # 2048
P = 128
KT = D // P                        # 32 k-tiles
MG = 2                             # row groups per core
MG_ROWS = ROWS_PER_CORE // MG      # 1024
MT = MG_ROWS // P                  # 8 m-tiles (PSUM banks) per group
OC = 512                           # o-chunk width (one PSUM bank)
N_OC = D // OC                     # 8

F32 = mybir.dt.float32
BF16 = mybir.dt.bfloat16

N_WARMUP = 7


def split_wide_waits(nc, max_waits=1):
    """walrus in this container rejects >1 sync wait per instruction;
    move excess waits onto preceding same-engine NoOps."""
    n_split = 0
    for f in nc.m.functions:
        for bb in f.blocks:
            new_insts = []
            for inst in bb.instructions:
                si = getattr(inst, "sync_info", None)
                if si is not None and si.on_wait and len(si.on_wait) > max_waits:
                    waits = list(si.on_wait)
                    keep = waits[-max_waits:]
                    extra = waits[:-max_waits]
                    for i in range(0, len(extra), max_waits):
                        chunk = extra[i:i + max_waits]
                        nop = mybir.InstNoOp(
                            name=f"{inst.name}_wsplit{i}",
                            sync_info=mybir.SyncInfo(on_wait=chunk, on_update=[]),
                            bass_nofuse=True,
                            engine=inst.engine,
                        )
                        new_insts.append(nop)
                        n_split += 1
                    si.on_wait = keep
                new_insts.append(inst)
            bb.instructions[:] = new_insts
    return n_split


def build_program():
    nc = bass.Bass()
    xt = nc.declare_dram_parameter("xt", [D, ROWS_PER_CORE], BF16, isOutput=False)
    wt = nc.declare_dram_parameter("wt", [D, D], BF16, isOutput=False)
    out = nc.declare_dram_parameter("out", [ROWS_PER_CORE, D], F32, isOutput=True)

    with tile.TileContext(nc) as tc:
        with (
            tc.tile_pool(name="xpool", bufs=KT * MG) as xpool,
            tc.tile_pool(name="wpool", bufs=36) as wpool,
            tc.tile_pool(name="opool", bufs=12) as opool,
            tc.tile_pool(name="wupool", bufs=1) as wupool,
            tc.tile_pool(name="ppool", bufs=8, space="PSUM") as ppool,
        ):
            # HAM warmup: junk matmuls trip the PE clock gate to 8/8
            # while the first x tiles stream in.
            wu = wupool.tile([P, OC], BF16, tag="wu")
            nc.vector.memset(wu[:], 0.0)
            junk = ppool.tile([P, OC], F32, tag="acc", name="junk")
            for i in range(N_WARMUP):
                nc.tensor.matmul(
                    junk[:],
                    wu[:, :P],
                    wu[:],
                    start=(i == 0),
                    stop=(i == N_WARMUP - 1),
                )

            # x shard resident: 64 tiles [128, 1024], issued in the order
            # the first pass consumes them (mg-major, k-ascending).
            # mg0 goes on the gpsimd/scalar rings; W oc0 is hoisted onto
            # the sync ring first, with mg1's x behind it (per-ring FIFO
            # keeps the mg1 prefetch from stealing HBM bandwidth from
            # pass 0's just-in-time W stream).
            xtiles = [[None] * KT for _ in range(MG)]
            for k in range(KT):
                t = xpool.tile([P, MG_ROWS], BF16, tag="x")
                # half-tile per ring: both rings deliver each k in
                # lockstep, and the first 4 m-tiles' matmuls only need
                # the first half
                half = MG_ROWS // 2
                nc.gpsimd.dma_start(
                    t[:, 0:half], xt[k * P:(k + 1) * P, 0:half]
                )
                nc.scalar.dma_start(
                    t[:, half:MG_ROWS], xt[k * P:(k + 1) * P, half:MG_ROWS]
                )
                xtiles[0][k] = t

            wtiles0 = [None] * KT
            for k in range(KT):
                w = wpool.tile([P, OC], BF16, tag="wt")
                nc.sync.dma_start(w[:], wt[k * P:(k + 1) * P, 0:OC])
                wtiles0[k] = w

            for k in range(KT):
                t = xpool.tile([P, MG_ROWS], BF16, tag="x")
                nc.sync.dma_start(
                    t[:], xt[k * P:(k + 1) * P, MG_ROWS:2 * MG_ROWS]
                )
                xtiles[1][k] = t

            def half_pass(oc, mg, mts, wtiles, load_w):
                psums = [
                    ppool.tile([P, OC], F32, tag="acc",
                               name=f"ps_{oc}_{mg}_{mt}")
                    for mt in mts
                ]
                for k in range(KT):
                    if load_w:
                        w = wpool.tile([P, OC], BF16, tag="wt")
                        nc.sync.dma_start(
                            w[:],
                            wt[k * P:(k + 1) * P, oc * OC:(oc + 1) * OC],
                        )
                        wtiles[k] = w
                    xk = xtiles[mg][k]
                    for i, mt in enumerate(mts):
                        nc.tensor.matmul(
                            psums[i][:],
                            xk[:, mt * P:(mt + 1) * P],
                            wtiles[k][:],
                            start=(k == 0),
                            stop=(k == KT - 1),
                        )
                for i, mt in enumerate(mts):
                    ot = opool.tile([P, OC], F32, tag="ot")
                    nc.vector.tensor_copy(ot[:], psums[i][:])
                    # alternate HWDGE rings so the drain chain at
                    # pass boundaries / kernel tail is half as deep
                    eng = nc.scalar if mt % 2 == 0 else nc.sync
                    eng.dma_start(
                        out[mg * MG_ROWS + mt * P:
                            mg * MG_ROWS + (mt + 1) * P,
                            oc * OC:(oc + 1) * OC],
                        ot[:],
                    )

            for oc in range(N_OC):
                wtiles = wtiles0 if oc == 0 else [None] * KT
                for mg in range(MG):
                    last = (oc == N_OC - 1 and mg == MG - 1)
                    load_w = (mg == 0 and oc > 0)
                    if last:
                        # split the final pass so only half the PSUM
                        # drain chain remains after the last matmul
                        half_pass(oc, mg, list(range(MT // 2)), wtiles,
                                  load_w)
                        half_pass(oc, mg, list(range(MT // 2, MT)),
                                  wtiles, False)
                    else:
                        half_pass(oc, mg, list(range(MT)), wtiles, load_w)

    split_wide_waits(nc)
    return nc


_NC_CACHE = [None]


def kernel(x, weight, lora_A, lora_B):
    import ml_dtypes
    from concourse.bass_utils import run_bass_kernel_spmd

    bf16 = ml_dtypes.bfloat16

    x = np.asarray(x, dtype=np.float32)
    weight = np.asarray(weight, dtype=np.float32)
    lora_A = np.asarray(lora_A, dtype=np.float32)
    lora_B = np.asarray(lora_B, dtype=np.float32)

    # fold LoRA: out = x @ (W.T + 2*B@A)
    wfold = weight.T + 2.0 * (lora_B @ lora_A)
    wt = np.ascontiguousarray(wfold.astype(bf16))

    x2 = x.reshape(ROWS_TOTAL, D)
    xt_all = x2.T.astype(bf16)     # [D, ROWS_TOTAL] C-contiguous

    in_maps = []
    for c in range(N_CORES):
        xt_c = np.ascontiguousarray(
            xt_all[:, c * ROWS_PER_CORE:(c + 1) * ROWS_PER_CORE]
        )
        in_maps.append({"xt": xt_c, "wt": wt})

    if _NC_CACHE[0] is None:
        _NC_CACHE[0] = build_program()
    nc = _NC_CACHE[0]

    res = run_bass_kernel_spmd(nc, in_maps, list(range(N_CORES)))
    out = np.concatenate(
        [res.results[c]["out"] for c in range(N_CORES)], axis=0
    )
    return out.reshape(x.shape)
